# revision 2
# baseline (speedup 1.0000x reference)
"""Causal multi-head attention on 8 TRN2 NeuronCores — v2.

Problem: x[4, 2048, 768], 12 heads x d_head 64, causal softmax attention.
Sharding: core c handles batch b = c//2 and the 6-head group h0 = 6*(c%2);
host sums the two half-outputs per batch.

v2 layout (per core) — same math as v1, restructured schedule:
  - pair-outer phase D with projections of pair p+1 and the output
    projection woven into the PE stream, so the ACT-engine exp time
    (~120us) hides under the PE matmul stream instead of serializing.
  - t-single PSUM layout: pss[128,1024] x2 (4 banks) + psz[65,512] x2
    (2 banks) + aux[128,512] x2 (2 banks, shared by projections and the
    output projection) = exactly 8 banks.
  - softmax normalize: denominator rows go through one DRAM bounce per
    (pair, t) — [1,1024] f32 -> [128,8] spread -> DVE reciprocal ->
    [1,1024] bf16 -> [64,1024] broadcast — 4 sync DMAs instead of v1's
    per-(par,t) chains that saturated the HWDGE path.
  - diagonal exps merged into one 3D-AP ACTIVATE per (t, r) covering both
    heads; off-diagonal exps are [128,1024] as before.
  - output written as bf16 (halves DMA bytes), one DMA per 128-row block.
"""

import sys

if "/opt/trn_rl_repo" not in sys.path:
    sys.path.insert(0, "/opt/trn_rl_repo")

import numpy as np
import ml_dtypes

BF16NP = ml_dtypes.bfloat16


def _ensure_ntff_hook():
    import types
    if "antenv.axon_hooks" in sys.modules:
        return
    try:
        from trn_agent_boot.trn_boot import _ntff_profile_via_ctypes
        hook = _ntff_profile_via_ctypes("/opt/axon/libaxon_pjrt.so")
    except Exception:
        hook = None
    m = types.ModuleType("antenv.axon_hooks")
    m._hook = hook
    m.get_axon_ntff_profile_hook = lambda: m._hook
    def _set(h):
        m._hook = h
    m.set_axon_ntff_profile_hook = _set
    sys.modules["antenv.axon_hooks"] = m


_ensure_ntff_hook()

import concourse.bass as bass
import concourse.tile as tile
from concourse import bacc, mybir
from concourse.bass_utils import run_bass_kernel_spmd

F32 = mybir.dt.float32
BF16 = mybir.dt.bfloat16
AF = mybir.ActivationFunctionType

D = 768          # d_model
S = 2048         # seq
E = 64           # d_head
NHC = 6          # heads per core
HE = NHC * E     # 384
KD = D // 128    # 6 k-chunks over d_model
B = 4

LAST_EXEC_TIME_NS = None
_GRAPH_CACHE = {}
DEBUG_ZT = False


def _build_graph(qkv_bias: bool) -> bass.Bass:
    nc = bacc.Bacc("TRN2", target_bir_lowering=False)
    xt = nc.declare_dram_parameter("xt", [D, S], BF16, isOutput=False)
    wq = nc.declare_dram_parameter("wq", [D, HE], BF16, isOutput=False)
    wk = nc.declare_dram_parameter("wk", [D, HE], BF16, isOutput=False)
    wv = nc.declare_dram_parameter("wv", [D, HE], BF16, isOutput=False)
    wo = nc.declare_dram_parameter("wo", [HE, D], BF16, isOutput=False)
    mask = nc.declare_dram_parameter("mask", [128, 128], BF16, isOutput=False)
    if qkv_bias:
        bq = nc.declare_dram_parameter("bq", [HE, 1], F32, isOutput=False)
        bk = nc.declare_dram_parameter("bk", [HE, 1], F32, isOutput=False)
        bv = nc.declare_dram_parameter("bv", [1, HE], BF16, isOutput=False)
    out = nc.declare_dram_parameter("out", [S, D], BF16, isOutput=True)
    if DEBUG_ZT:
        zdbg = nc.declare_dram_parameter("zdbg", [HE, S], BF16, isOutput=True)
        ndbg = nc.declare_dram_parameter("ndbg", [65 * 2 + 64, 1024], BF16,
                                         isOutput=True)

    with tile.TileContext(nc) as tc:
        with tc.tile_pool(name="persist", bufs=1) as persist, \
             tc.tile_pool(name="workE", bufs=12) as workE, \
             tc.tile_pool(name="workZ", bufs=10) as workZ, \
             tc.tile_pool(name="work2", bufs=8) as work2, \
             tc.tile_pool(name="workO", bufs=3) as workO, \
             tc.tile_pool(name="dramP", bufs=3, space="DRAM") as dramP, \
             tc.tile_pool(name="psS", bufs=2, space="PSUM") as psS, \
             tc.tile_pool(name="psZ", bufs=1, space="PSUM") as psZ, \
             tc.tile_pool(name="aux", bufs=2, space="PSUM") as auxP:

            QT = [persist.tile([128, S], BF16, tag=f"qt{m}", name=f"qt{m}") for m in range(3)]
            KT = [persist.tile([128, S], BF16, tag=f"kt{m}", name=f"kt{m}") for m in range(3)]
            ZT = [persist.tile([128, S], BF16, tag=f"zt{m}", name=f"zt{m}") for m in range(3)]
            VA = [persist.tile([128, NHC * 65], BF16, tag=f"va{s}", name=f"va{s}") for s in range(16)]
            WO = [persist.tile([128, D], BF16, tag=f"wo{m}", name=f"wo{m}") for m in range(3)]
            MSK = persist.tile([128, 128], BF16, tag="mask", name="mask_sb")
            XT = [persist.tile([128, S], BF16, tag=f"xt{k}", name=f"xt{k}") for k in range(KD)]
            WQs = [persist.tile([128, HE], BF16, tag=f"wq{k}", name=f"wq{k}") for k in range(KD)]
            WKs = [persist.tile([128, HE], BF16, tag=f"wk{k}", name=f"wk{k}") for k in range(KD)]
            WVs = [persist.tile([128, HE], BF16, tag=f"wv{k}", name=f"wv{k}") for k in range(KD)]

            # loads, ordered so the first projection matmuls can start early
            for k in range(KD):
                nc.sync.dma_start(out=WQs[k][:], in_=wq[k * 128:(k + 1) * 128, :])
                nc.sync.dma_start(out=XT[k][:], in_=xt[k * 128:(k + 1) * 128, :])
                nc.sync.dma_start(out=WKs[k][:], in_=wk[k * 128:(k + 1) * 128, :])
                nc.sync.dma_start(out=WVs[k][:], in_=wv[k * 128:(k + 1) * 128, :])
            nc.sync.dma_start(out=MSK[:], in_=mask[:])
            for m in range(3):
                nc.sync.dma_start(out=WO[m][:], in_=wo[m * 128:(m + 1) * 128, :])
            ONES = persist.tile([1, 128], BF16, tag="ones", name="ones_sb")
            nc.vector.memset(ONES[:], 1.0)
            if qkv_bias:
                BQ = persist.tile([128, 3], F32, tag="bq", name="bq_sb")
                BK = persist.tile([128, 3], F32, tag="bk", name="bk_sb")
                BV = persist.tile([1, HE], BF16, tag="bv", name="bv_sb")
                for m in range(3):
                    nc.sync.dma_start(out=BQ[:, m:m + 1], in_=bq[m * 128:(m + 1) * 128, :])
                    nc.sync.dma_start(out=BK[:, m:m + 1], in_=bk[m * 128:(m + 1) * 128, :])
                nc.sync.dma_start(out=BV[:], in_=bv[:])

            # ---------- emission helpers ----------
            def proj_qk(hp, n):
                """Q^T and K^T slice [128, 512] for pair hp, q-supertile n."""
                for Wt, Ot, bt in ((WQs, QT, "bq"), (WKs, KT, "bk")):
                    ps = auxP.tile([128, 512], F32, tag="aux", name="ps_aux")
                    for k in range(KD):
                        nc.tensor.matmul(
                            ps[:],
                            Wt[k][:, hp * 128:(hp + 1) * 128],
                            XT[k][:, n * 512:(n + 1) * 512],
                            start=(k == 0), stop=(k == KD - 1))
                    dst = Ot[hp][:, n * 512:(n + 1) * 512]
                    if qkv_bias:
                        bias_t = BQ if bt == "bq" else BK
                        nc.scalar.activation(dst, ps[:], AF.Copy,
                                             bias=bias_t[:, hp:hp + 1])
                    else:
                        nc.vector.tensor_copy(dst, ps[:])

            v_done = [False] * 16

            def proj_v(sc):
                """V rows for token block sc, all 6 heads, into VA[sc]."""
                if v_done[sc]:
                    return
                v_done[sc] = True
                nc.vector.memset(VA[sc][:], 1.0)
                ps = auxP.tile([128, 512], F32, tag="aux", name="ps_aux")
                pv = ps[:, 0:HE]
                for k in range(KD):
                    nc.tensor.matmul(
                        pv,
                        XT[k][:, sc * 128:(sc + 1) * 128],
                        WVs[k][:],
                        start=(k == 0), stop=False if qkv_bias else (k == KD - 1))
                if qkv_bias:
                    nc.tensor.matmul(pv, ONES[:], BV[:], start=False, stop=True)
                nc.vector.tensor_copy(
                    VA[sc][:].rearrange("p (h c) -> p h c", c=65)[:, :, 0:64],
                    pv.rearrange("p (h c) -> p h c", c=64))

            def phase_e(t):
                """Output projection for the 4 token blocks of supertile t."""
                for mc in range(4 * t, 4 * t + 4):
                    ob = workO.tile([128, D], BF16, tag="ob", name="ob")
                    for half in range(2):
                        po = auxP.tile([128, 512], F32, tag="aux", name="po")
                        pon = po[:, 0:HE]
                        n0 = half * HE
                        for k in range(3):
                            nc.tensor.matmul(
                                pon,
                                ZT[k][:, mc * 128:(mc + 1) * 128],
                                WO[k][:, n0:n0 + HE],
                                start=(k == 0), stop=(k == 2))
                        nc.vector.tensor_copy(ob[:, n0:n0 + HE], pon)
                    nc.sync.dma_start(out=out[mc * 128:(mc + 1) * 128, :], in_=ob[:])

            # ---------- phase D ----------
            pending = []   # deferred normalize tails
            av_q = []      # deferred AV matmuls: (av_fn, norm_fn|None)
            AV_LAG = 8

            def drain_pending(upto):
                while len(pending) > upto:
                    pending.pop(0)()

            def pump_avs(lag):
                while len(av_q) > lag:
                    av_fn, norm_fn = av_q.pop(0)
                    av_fn()
                    if norm_fn is not None:
                        norm_fn()
                        drain_pending(2)

            HOIST_V = False
            for hp in range(3):
                if hp == 0:
                    for n in range(4):
                        proj_qk(0, n)
                    if HOIST_V:
                        for sc in range(16):
                            proj_v(sc)
                for t in range(4):
                    if hp < 2:
                        proj_qk(hp + 1, t)
                    if hp == 2 and t >= 1:
                        pump_avs(0)
                        drain_pending(0)
                        phase_e(t - 1)
                    psz = {}

                    def emit_normalize(t=t, hp=hp, psz=psz):
                        """Both heads' psz done: drain, reciprocal, scale."""
                        zraw = {}
                        dd = dramP.tile([1, 1024], BF16, tag="dd", name="dd")
                        dd_ap = dd[:]
                        for par in (0, 1):
                            zr = workZ.tile([65, 512], BF16, tag="zraw", name="zraw")
                            nc.vector.tensor_copy(zr[:], psz[par][:])
                            nc.sync.dma_start(
                                out=dd[0:1, par * 512:(par + 1) * 512],
                                in_=zr[64:65, :])
                            zraw[par] = zr
                        rp = work2.tile([128, 8], BF16, tag="rp", name="rp")
                        nc.sync.dma_start(out=rp[:], in_=bass.AP(
                            tensor=dd_ap.tensor, offset=dd_ap.offset,
                            ap=[[8, 128], [1, 8]]))
                        rcp = work2.tile([128, 8], BF16, tag="rcp", name="rcp")
                        with nc.allow_low_precision(reason="softmax recip bf16"):
                            nc.vector.reciprocal(rcp[:], rp[:])
                        rcd = dramP.tile([1, 1024], BF16, tag="rcd", name="rcd")
                        rcd_ap = rcd[:]
                        nc.sync.dma_start(out=bass.AP(
                            tensor=rcd_ap.tensor, offset=rcd_ap.offset,
                            ap=[[8, 128], [1, 8]]), in_=rcp[:])
                        bc = work2.tile([64, 1024], BF16, tag="bc", name="bc")
                        nc.sync.dma_start(out=bc[:], in_=bass.AP(
                            tensor=rcd_ap.tensor, offset=rcd_ap.offset,
                            ap=[[0, 64], [1, 1024]]))

                        def part2(zraw=zraw, bc=bc, t=t, hp=hp):
                            for par in (0, 1):
                                nc.vector.tensor_mul(
                                    ZT[hp][par * 64:par * 64 + 64,
                                           t * 512:(t + 1) * 512],
                                    zraw[par][0:64, :],
                                    bc[:, par * 512:(par + 1) * 512])
                        pending.append(part2)

                    for j in range(4 * t + 4):
                        if hp == 0:
                            proj_v(j)
                        r = j - 4 * t
                        q0 = 128 * r if r >= 0 else 0
                        pss = psS.tile([128, 1024], F32, tag="pss", name="pss")
                        for par in (0, 1):
                            nc.tensor.matmul(
                                pss[:, par * 512 + q0:par * 512 + 512],
                                KT[hp][par * 64:par * 64 + 64, j * 128:(j + 1) * 128],
                                QT[hp][par * 64:par * 64 + 64,
                                       t * 512 + q0:(t + 1) * 512],
                                start=True, stop=True)
                        et = workE.tile([128, 1024], BF16, tag="et", name="et")
                        if r >= 0:
                            nw = 512 - q0
                            src = pss[:].rearrange("p (g q) -> p g q", g=2)[:, :, q0:512]
                            dst = et[:].rearrange("p (g q) -> p g q", g=2)[:, :, q0:512]
                            nc.scalar.activation(dst, src, AF.Exp, scale=0.125)
                            for par in (0, 1):
                                nc.vector.tensor_mul(
                                    et[:, par * 512 + q0:par * 512 + q0 + 128],
                                    et[:, par * 512 + q0:par * 512 + q0 + 128],
                                    MSK[:])
                        else:
                            nc.scalar.activation(et[:], pss[:], AF.Exp, scale=0.125)
                        for par in (0, 1):
                            def av_fn(par=par, et=et, j=j, t=t, q0=q0, psz=psz,
                                      hp=hp):
                                if j == 0:
                                    psz[par] = psZ.tile(
                                        [65, 512], F32, tag=f"pz{par}",
                                        name=f"pz{par}")
                                h = 2 * hp + par
                                nc.tensor.matmul(
                                    psz[par][:, q0:512],
                                    VA[j][:, h * 65:(h + 1) * 65],
                                    et[:, par * 512 + q0:par * 512 + 512],
                                    start=(j == 0), stop=(j == 4 * t + 3))
                            norm_fn = (emit_normalize
                                       if (j == 4 * t + 3 and par == 1) else None)
                            av_q.append((av_fn, norm_fn))
                        pump_avs(AV_LAG)
            pump_avs(0)
            drain_pending(0)
            phase_e(3)
            if DEBUG_ZT:
                for m in range(3):
                    nc.sync.dma_start(out=zdbg[m * 128:(m + 1) * 128, :],
                                      in_=ZT[m][:])
                nc.sync.dma_start(out=ndbg[0:128, 0:390], in_=VA[12][:])
                nc.sync.dma_start(out=ndbg[0:128, 390:780], in_=VA[8][:])
    nc.compile()
    return nc


def _build_mask() -> np.ndarray:
    kl = np.arange(128)[:, None]
    ql = np.arange(128)[None, :]
    return (ql >= kl).astype(np.float32)


def kernel(**inputs) -> np.ndarray:
    global LAST_EXEC_TIME_NS
    x = np.asarray(inputs["normalized_resid_pre"], dtype=np.float32)
    W_Q = np.asarray(inputs["W_Q"], dtype=np.float32)
    W_K = np.asarray(inputs["W_K"], dtype=np.float32)
    W_V = np.asarray(inputs["W_V"], dtype=np.float32)
    W_O = np.asarray(inputs["W_O"], dtype=np.float32)
    b_Q = np.asarray(inputs["b_Q"], dtype=np.float32)
    b_K = np.asarray(inputs["b_K"], dtype=np.float32)
    b_V = np.asarray(inputs["b_V"], dtype=np.float32)
    b_O = np.asarray(inputs["b_O"], dtype=np.float32)

    qkv_bias = bool(b_Q.any() or b_K.any() or b_V.any())
    key = qkv_bias
    if key not in _GRAPH_CACHE:
        _GRAPH_CACHE[key] = _build_graph(qkv_bias)
    nc = _GRAPH_CACHE[key]

    mask = _build_mask()
    in_maps = []
    for c in range(8):
        b, h0 = c // 2, NHC * (c % 2)
        im = {
            "xt": np.ascontiguousarray(x[b].T).astype(BF16NP),
            "wq": np.ascontiguousarray(
                W_Q[h0:h0 + NHC].transpose(1, 0, 2).reshape(D, HE)).astype(BF16NP),
            "wk": np.ascontiguousarray(
                W_K[h0:h0 + NHC].transpose(1, 0, 2).reshape(D, HE)).astype(BF16NP),
            "wv": np.ascontiguousarray(
                W_V[h0:h0 + NHC].transpose(1, 0, 2).reshape(D, HE)).astype(BF16NP),
            "wo": np.ascontiguousarray(W_O[h0:h0 + NHC].reshape(HE, D)).astype(BF16NP),
            "mask": mask.astype(BF16NP),
        }
        if qkv_bias:
            im["bq"] = np.ascontiguousarray(b_Q[h0:h0 + NHC].reshape(HE, 1))
            im["bk"] = np.ascontiguousarray(b_K[h0:h0 + NHC].reshape(HE, 1))
            im["bv"] = np.ascontiguousarray(b_V[h0:h0 + NHC].reshape(1, HE)).astype(BF16NP)
        in_maps.append(im)

    import os
    trace = bool(os.environ.get("KERNEL_TRACE"))
    res = run_bass_kernel_spmd(nc, in_maps, core_ids=list(range(8)), trace=trace)
    LAST_EXEC_TIME_NS = res.exec_time_ns
    results = res.results

    outf = np.empty((B, S, D), dtype=np.float32)
    for b in range(B):
        outf[b] = results[2 * b]["out"].astype(np.float32) + \
            results[2 * b + 1]["out"].astype(np.float32)
    if b_O.any():
        outf += b_O
    return outf


# revision 3
# speedup vs baseline: 1.0186x; 1.0186x over previous
"""Causal multi-head attention on 8 TRN2 NeuronCores — v2.

Problem: x[4, 2048, 768], 12 heads x d_head 64, causal softmax attention.
Sharding: core c handles batch b = c//2 and the 6-head group h0 = 6*(c%2);
host sums the two half-outputs per batch.

v2 layout (per core) — same math as v1, restructured schedule:
  - pair-outer phase D with projections of pair p+1 and the output
    projection woven into the PE stream, so the ACT-engine exp time
    (~120us) hides under the PE matmul stream instead of serializing.
  - t-single PSUM layout: pss[128,1024] x2 (4 banks) + psz[65,512] x2
    (2 banks) + aux[128,512] x2 (2 banks, shared by projections and the
    output projection) = exactly 8 banks.
  - softmax normalize: denominator rows go through one DRAM bounce per
    (pair, t) — [1,1024] f32 -> [128,8] spread -> DVE reciprocal ->
    [1,1024] bf16 -> [64,1024] broadcast — 4 sync DMAs instead of v1's
    per-(par,t) chains that saturated the HWDGE path.
  - diagonal exps merged into one 3D-AP ACTIVATE per (t, r) covering both
    heads; off-diagonal exps are [128,1024] as before.
  - output written as bf16 (halves DMA bytes), one DMA per 128-row block.
"""

import sys

if "/opt/trn_rl_repo" not in sys.path:
    sys.path.insert(0, "/opt/trn_rl_repo")

import numpy as np
import ml_dtypes

BF16NP = ml_dtypes.bfloat16


def _ensure_ntff_hook():
    import types
    if "antenv.axon_hooks" in sys.modules:
        return
    try:
        from trn_agent_boot.trn_boot import _ntff_profile_via_ctypes
        hook = _ntff_profile_via_ctypes("/opt/axon/libaxon_pjrt.so")
    except Exception:
        hook = None
    m = types.ModuleType("antenv.axon_hooks")
    m._hook = hook
    m.get_axon_ntff_profile_hook = lambda: m._hook
    def _set(h):
        m._hook = h
    m.set_axon_ntff_profile_hook = _set
    sys.modules["antenv.axon_hooks"] = m


_ensure_ntff_hook()

import concourse.bass as bass
import concourse.tile as tile
from concourse import bacc, mybir, library_config
from concourse.bass_utils import run_bass_kernel_spmd

F32 = mybir.dt.float32
BF16 = mybir.dt.bfloat16
AF = mybir.ActivationFunctionType

D = 768          # d_model
S = 2048         # seq
E = 64           # d_head
NHC = 6          # heads per core
HE = NHC * E     # 384
KD = D // 128    # 6 k-chunks over d_model
B = 4

LAST_EXEC_TIME_NS = None
_GRAPH_CACHE = {}
DEBUG_ZT = False


def _build_graph(qkv_bias: bool) -> bass.Bass:
    nc = bacc.Bacc("TRN2", target_bir_lowering=False)
    xt = nc.declare_dram_parameter("xt", [D, S], BF16, isOutput=False)
    wq = nc.declare_dram_parameter("wq", [D, HE], BF16, isOutput=False)
    wk = nc.declare_dram_parameter("wk", [D, HE], BF16, isOutput=False)
    wv = nc.declare_dram_parameter("wv", [D, HE], BF16, isOutput=False)
    wo = nc.declare_dram_parameter("wo", [HE, D], BF16, isOutput=False)
    mask = nc.declare_dram_parameter("mask", [128, 128], BF16, isOutput=False)
    if qkv_bias:
        bq = nc.declare_dram_parameter("bq", [HE, 1], F32, isOutput=False)
        bk = nc.declare_dram_parameter("bk", [HE, 1], F32, isOutput=False)
        bv = nc.declare_dram_parameter("bv", [1, HE], BF16, isOutput=False)
    out = nc.declare_dram_parameter("out", [S, D], BF16, isOutput=True)
    if DEBUG_ZT:
        zdbg = nc.declare_dram_parameter("zdbg", [HE, S], BF16, isOutput=True)
        ndbg = nc.declare_dram_parameter("ndbg", [65 * 2 + 64, 1024], BF16,
                                         isOutput=True)

    with tile.TileContext(nc) as tc:
        with tc.tile_pool(name="persist", bufs=1) as persist, \
             tc.tile_pool(name="workE", bufs=12) as workE, \
             tc.tile_pool(name="workZ", bufs=6) as workZ, \
             tc.tile_pool(name="work2", bufs=3) as work2, \
             tc.tile_pool(name="workO", bufs=3) as workO, \
             tc.tile_pool(name="dramP", bufs=3, space="DRAM") as dramP, \
             tc.tile_pool(name="psS", bufs=2, space="PSUM") as psS, \
             tc.tile_pool(name="psZ", bufs=1, space="PSUM") as psZ, \
             tc.tile_pool(name="aux", bufs=2, space="PSUM") as auxP:

            QT = [persist.tile([128, S], BF16, tag=f"qt{m}", name=f"qt{m}") for m in range(3)]
            KT = [persist.tile([128, S], BF16, tag=f"kt{m}", name=f"kt{m}") for m in range(3)]
            ZT = [persist.tile([128, S], BF16, tag=f"zt{m}", name=f"zt{m}") for m in range(3)]
            VA = [persist.tile([128, NHC * 65], BF16, tag=f"va{s}", name=f"va{s}") for s in range(16)]
            WO = [persist.tile([128, D], BF16, tag=f"wo{m}", name=f"wo{m}") for m in range(3)]
            MSK = persist.tile([128, 128], BF16, tag="mask", name="mask_sb")
            XT = [persist.tile([128, S], BF16, tag=f"xt{k}", name=f"xt{k}") for k in range(KD)]
            WQs = [persist.tile([128, HE], BF16, tag=f"wq{k}", name=f"wq{k}") for k in range(KD)]
            WKs = [persist.tile([128, HE], BF16, tag=f"wk{k}", name=f"wk{k}") for k in range(KD)]
            WVs = [persist.tile([128, HE], BF16, tag=f"wv{k}", name=f"wv{k}") for k in range(KD)]

            # loads, ordered so the first projection matmuls can start early:
            # k-outer projections only need chunk k of (wq, wk, xt) at a time
            nc.gpsimd.load_library(library_config.attn)

            # HAM warm-up: the PE idles during input DMA anyway; a burst of
            # concurrent dummy matmul pairs (disjoint row groups, ~6us) flips
            # the PE clock gate to 8/8 (2.4 GHz) before the real stream starts.
            WUP = persist.tile([128, 512], BF16, tag="wup", name="wup")
            nc.vector.memset(WUP[:], 0.0)
            for i in range(16):
                pz0 = psZ.tile([65, 512], F32, tag="pz0", name="pz0")
                pz1 = psZ.tile([65, 512], F32, tag="pz1", name="pz1")
                nc.tensor.matmul(pz0[0:64, :], WUP[0:64, 0:64], WUP[0:64, :],
                                 start=True, stop=True, tile_position=(0, 0))
                nc.tensor.matmul(pz1[0:64, :], WUP[64:128, 0:64], WUP[64:128, :],
                                 start=True, stop=True, tile_position=(64, 0))
            for k in range(KD):
                nc.sync.dma_start(out=WQs[k][:], in_=wq[k * 128:(k + 1) * 128, :])
                nc.sync.dma_start(out=WKs[k][:], in_=wk[k * 128:(k + 1) * 128, :])
                nc.sync.dma_start(out=XT[k][:], in_=xt[k * 128:(k + 1) * 128, :])
            for k in range(KD):
                nc.sync.dma_start(out=WVs[k][:], in_=wv[k * 128:(k + 1) * 128, :])
            nc.sync.dma_start(out=MSK[:], in_=mask[:])
            for m in range(3):
                nc.sync.dma_start(out=WO[m][:], in_=wo[m * 128:(m + 1) * 128, :])
            ONES = persist.tile([1, 128], BF16, tag="ones", name="ones_sb")
            nc.vector.memset(ONES[:], 1.0)
            if qkv_bias:
                BQ = persist.tile([128, 3], F32, tag="bq", name="bq_sb")
                BK = persist.tile([128, 3], F32, tag="bk", name="bk_sb")
                BV = persist.tile([1, HE], BF16, tag="bv", name="bv_sb")
                for m in range(3):
                    nc.sync.dma_start(out=BQ[:, m:m + 1], in_=bq[m * 128:(m + 1) * 128, :])
                    nc.sync.dma_start(out=BK[:, m:m + 1], in_=bk[m * 128:(m + 1) * 128, :])
                nc.sync.dma_start(out=BV[:], in_=bv[:])

            # ---------- emission helpers ----------
            def proj_qk(hp, n):
                """Q^T and K^T slice [128, 512] for pair hp, q-supertile n."""
                for Wt, Ot, bt in ((WQs, QT, "bq"), (WKs, KT, "bk")):
                    ps = auxP.tile([128, 512], F32, tag="aux", name="ps_aux")
                    for k in range(KD):
                        nc.tensor.matmul(
                            ps[:],
                            Wt[k][:, hp * 128:(hp + 1) * 128],
                            XT[k][:, n * 512:(n + 1) * 512],
                            start=(k == 0), stop=(k == KD - 1))
                    dst = Ot[hp][:, n * 512:(n + 1) * 512]
                    if qkv_bias:
                        bias_t = BQ if bt == "bq" else BK
                        nc.scalar.activation(dst, ps[:], AF.Copy,
                                             bias=bias_t[:, hp:hp + 1])
                    else:
                        nc.vector.tensor_copy(dst, ps[:])

            v_done = [False] * 16

            def proj_v(sc):
                """V rows for token block sc, all 6 heads, into VA[sc]."""
                if v_done[sc]:
                    return
                v_done[sc] = True
                nc.vector.memset(VA[sc][:], 1.0)
                ps = auxP.tile([128, 512], F32, tag="aux", name="ps_aux")
                pv = ps[:, 0:HE]
                for k in range(KD):
                    nc.tensor.matmul(
                        pv,
                        XT[k][:, sc * 128:(sc + 1) * 128],
                        WVs[k][:],
                        start=(k == 0), stop=False if qkv_bias else (k == KD - 1))
                if qkv_bias:
                    nc.tensor.matmul(pv, ONES[:], BV[:], start=False, stop=True)
                nc.vector.tensor_copy(
                    VA[sc][:].rearrange("p (h c) -> p h c", c=65)[:, :, 0:64],
                    pv.rearrange("p (h c) -> p h c", c=64))

            def phase_e(t):
                """Output projection for the 4 token blocks of supertile t."""
                for mc in range(4 * t, 4 * t + 4):
                    ob = workO.tile([128, D], BF16, tag="ob", name="ob")
                    for half in range(2):
                        po = auxP.tile([128, 512], F32, tag="aux", name="po")
                        pon = po[:, 0:HE]
                        n0 = half * HE
                        for k in range(3):
                            nc.tensor.matmul(
                                pon,
                                ZT[k][:, mc * 128:(mc + 1) * 128],
                                WO[k][:, n0:n0 + HE],
                                start=(k == 0), stop=(k == 2))
                        nc.vector.tensor_copy(ob[:, n0:n0 + HE], pon)
                    nc.sync.dma_start(out=out[mc * 128:(mc + 1) * 128, :], in_=ob[:])

            # ---------- phase D ----------
            pending = []   # deferred normalize tails
            av_q = []      # deferred AV matmuls: (av_fn, norm_fn|None)
            AV_LAG = 8

            def drain_pending(upto):
                while len(pending) > upto:
                    pending.pop(0)()

            def pump_avs(lag):
                while len(av_q) > lag:
                    av_fn, norm_fn = av_q.pop(0)
                    av_fn()
                    if norm_fn is not None:
                        norm_fn()
                        drain_pending(2)

            def proj_qk0_kouter():
                """Pair-0 Q (all 4 supertiles) + K (n=0,1) with the k-chunk
                loop OUTERMOST, so each arriving (wq[k], wk[k], xt[k]) chunk
                immediately unlocks 6 matmuls — the PE starts ~2us into the
                input loads instead of waiting for all of them."""
                pq = [psS.tile([128, 1024], F32, tag="pss", name="ps_q")
                      for _ in range(2)]
                pk = [auxP.tile([128, 512], F32, tag="aux", name="ps_k")
                      for _ in range(2)]
                for k in range(KD):
                    for n in range(4):
                        nc.tensor.matmul(
                            pq[n // 2][:, (n % 2) * 512:(n % 2) * 512 + 512],
                            WQs[k][:, 0:128],
                            XT[k][:, n * 512:(n + 1) * 512],
                            start=(k == 0), stop=(k == KD - 1))
                    for n in range(2):
                        nc.tensor.matmul(
                            pk[n][:],
                            WKs[k][:, 0:128],
                            XT[k][:, n * 512:(n + 1) * 512],
                            start=(k == 0), stop=(k == KD - 1))
                for i in range(2):
                    if qkv_bias:
                        nc.scalar.activation(QT[0][:, i * 1024:(i + 1) * 1024],
                                             pq[i][:], AF.Copy, bias=BQ[:, 0:1])
                        nc.scalar.activation(KT[0][:, i * 512:(i + 1) * 512],
                                             pk[i][:], AF.Copy, bias=BK[:, 0:1])
                    else:
                        nc.vector.tensor_copy(QT[0][:, i * 1024:(i + 1) * 1024],
                                              pq[i][:])
                        nc.vector.tensor_copy(KT[0][:, i * 512:(i + 1) * 512],
                                              pk[i][:])
                # K supertiles 2,3 (needed only from t=2 onward)
                for n in (2, 3):
                    ps = auxP.tile([128, 512], F32, tag="aux", name="ps_aux")
                    for k in range(KD):
                        nc.tensor.matmul(
                            ps[:], WKs[k][:, 0:128],
                            XT[k][:, n * 512:(n + 1) * 512],
                            start=(k == 0), stop=(k == KD - 1))
                    dst = KT[0][:, n * 512:(n + 1) * 512]
                    if qkv_bias:
                        nc.scalar.activation(dst, ps[:], AF.Copy, bias=BK[:, 0:1])
                    else:
                        nc.vector.tensor_copy(dst, ps[:])

            for hp in range(3):
                if hp == 0:
                    proj_qk0_kouter()
                t_order = (3, 0, 1, 2) if hp == 2 else (0, 1, 2, 3)
                for ti, t in enumerate(t_order):
                    if hp < 2:
                        proj_qk(hp + 1, t)
                    if hp == 2 and ti >= 1:
                        pump_avs(0)
                        drain_pending(0)
                        phase_e(t_order[ti - 1])
                    psz = {}

                    def emit_normalize(t=t, hp=hp, psz=psz):
                        """Both heads' psz done: drain, reciprocal, scale.
                        psz row 0 = denominator (ones-first VA layout), so the
                        whole chain is on-chip: DVE recip + gpsimd broadcast."""
                        zraw = {}
                        den = work2.tile([1, 1024], F32, tag="den", name="den")
                        for par in (0, 1):
                            zr = workZ.tile([64, 512], BF16, tag="zraw", name="zraw")
                            nc.vector.tensor_copy(zr[:], psz[par][0:64, :])
                            nc.vector.tensor_copy(
                                den[:, par * 512:(par + 1) * 512],
                                psz[par][64:65, :])
                            zraw[par] = zr
                        rcp = work2.tile([1, 1024], F32, tag="rcp", name="rcp")
                        nc.vector.reciprocal_approx_fast(rcp[:], den[:])
                        bc = work2.tile([64, 1024], F32, tag="bc", name="bc")
                        nc.gpsimd.partition_broadcast(bc[:], rcp[:])

                        def part2(zraw=zraw, bc=bc, t=t, hp=hp):
                            for par in (0, 1):
                                nc.vector.tensor_mul(
                                    ZT[hp][par * 64:par * 64 + 64,
                                           t * 512:(t + 1) * 512],
                                    zraw[par][:],
                                    bc[:, par * 512:(par + 1) * 512])
                        pending.append(part2)

                    for j in range(4 * t + 4):
                        if hp == 0:
                            proj_v(j)
                        r = j - 4 * t
                        q0 = 128 * r if r >= 0 else 0
                        pss = psS.tile([128, 1024], F32, tag="pss", name="pss")
                        for par in (0, 1):
                            nc.tensor.matmul(
                                pss[:, par * 512 + q0:par * 512 + 512],
                                KT[hp][par * 64:par * 64 + 64, j * 128:(j + 1) * 128],
                                QT[hp][par * 64:par * 64 + 64,
                                       t * 512 + q0:(t + 1) * 512],
                                start=True, stop=True)
                        et = workE.tile([128, 1024], BF16, tag="et", name="et")
                        if r >= 0:
                            nw = 512 - q0
                            src = pss[:].rearrange("p (g q) -> p g q", g=2)[:, :, q0:512]
                            dst = et[:].rearrange("p (g q) -> p g q", g=2)[:, :, q0:512]
                            nc.scalar.activation(dst, src, AF.Exp, scale=0.125)
                            for par in (0, 1):
                                nc.vector.tensor_mul(
                                    et[:, par * 512 + q0:par * 512 + q0 + 128],
                                    et[:, par * 512 + q0:par * 512 + q0 + 128],
                                    MSK[:])
                        else:
                            nc.scalar.activation(et[:], pss[:], AF.Exp, scale=0.125)
                        for par in (0, 1):
                            def av_fn(par=par, et=et, j=j, t=t, q0=q0, psz=psz,
                                      hp=hp):
                                if j == 0:
                                    psz[par] = psZ.tile(
                                        [65, 512], F32, tag=f"pz{par}",
                                        name=f"pz{par}")
                                h = 2 * hp + par
                                nc.tensor.matmul(
                                    psz[par][:, q0:512],
                                    VA[j][:, h * 65:(h + 1) * 65],
                                    et[:, par * 512 + q0:par * 512 + 512],
                                    start=(j == 0), stop=(j == 4 * t + 3))
                            norm_fn = (emit_normalize
                                       if (j == 4 * t + 3 and par == 1) else None)
                            av_q.append((av_fn, norm_fn))
                        pump_avs(AV_LAG)
            pump_avs(0)
            drain_pending(0)
            phase_e(2)
            if DEBUG_ZT:
                for m in range(3):
                    nc.sync.dma_start(out=zdbg[m * 128:(m + 1) * 128, :],
                                      in_=ZT[m][:])
                nc.sync.dma_start(out=ndbg[0:128, 0:390], in_=VA[12][:])
                nc.sync.dma_start(out=ndbg[0:128, 390:780], in_=VA[8][:])
    nc.compile()
    return nc


def _build_mask() -> np.ndarray:
    kl = np.arange(128)[:, None]
    ql = np.arange(128)[None, :]
    return (ql >= kl).astype(np.float32)


def kernel(**inputs) -> np.ndarray:
    global LAST_EXEC_TIME_NS
    x = np.asarray(inputs["normalized_resid_pre"], dtype=np.float32)
    W_Q = np.asarray(inputs["W_Q"], dtype=np.float32)
    W_K = np.asarray(inputs["W_K"], dtype=np.float32)
    W_V = np.asarray(inputs["W_V"], dtype=np.float32)
    W_O = np.asarray(inputs["W_O"], dtype=np.float32)
    b_Q = np.asarray(inputs["b_Q"], dtype=np.float32)
    b_K = np.asarray(inputs["b_K"], dtype=np.float32)
    b_V = np.asarray(inputs["b_V"], dtype=np.float32)
    b_O = np.asarray(inputs["b_O"], dtype=np.float32)

    qkv_bias = bool(b_Q.any() or b_K.any() or b_V.any())
    key = qkv_bias
    if key not in _GRAPH_CACHE:
        _GRAPH_CACHE[key] = _build_graph(qkv_bias)
    nc = _GRAPH_CACHE[key]

    mask = _build_mask()
    in_maps = []
    for c in range(8):
        b, h0 = c // 2, NHC * (c % 2)
        im = {
            "xt": np.ascontiguousarray(x[b].T).astype(BF16NP),
            "wq": np.ascontiguousarray(
                W_Q[h0:h0 + NHC].transpose(1, 0, 2).reshape(D, HE)).astype(BF16NP),
            "wk": np.ascontiguousarray(
                W_K[h0:h0 + NHC].transpose(1, 0, 2).reshape(D, HE)).astype(BF16NP),
            "wv": np.ascontiguousarray(
                W_V[h0:h0 + NHC].transpose(1, 0, 2).reshape(D, HE)).astype(BF16NP),
            "wo": np.ascontiguousarray(W_O[h0:h0 + NHC].reshape(HE, D)).astype(BF16NP),
            "mask": mask.astype(BF16NP),
        }
        if qkv_bias:
            im["bq"] = np.ascontiguousarray(b_Q[h0:h0 + NHC].reshape(HE, 1))
            im["bk"] = np.ascontiguousarray(b_K[h0:h0 + NHC].reshape(HE, 1))
            im["bv"] = np.ascontiguousarray(b_V[h0:h0 + NHC].reshape(1, HE)).astype(BF16NP)
        in_maps.append(im)

    import os
    trace = bool(os.environ.get("KERNEL_TRACE"))
    res = run_bass_kernel_spmd(nc, in_maps, core_ids=list(range(8)), trace=trace)
    LAST_EXEC_TIME_NS = res.exec_time_ns
    results = res.results

    outf = np.empty((B, S, D), dtype=np.float32)
    for b in range(B):
        outf[b] = results[2 * b]["out"].astype(np.float32) + \
            results[2 * b + 1]["out"].astype(np.float32)
    if b_O.any():
        outf += b_O
    return outf
